# revision 40
# baseline (speedup 1.0000x reference)
"""Bounding-box kernel for Trainium2 (Bass/Tile), 8-core SPMD.

Problem: mask [128, 1, 512, 512] f32 -> bbox [128, 4] int32
  (y_min, x_min, y_max, x_max) of the region where mask >= 0.5,
  with (0, 0, H, W) when a row/col has no hit.

Strategy (per core, 16 images, single qSync HWDGE queue):
  - DMA each image [512, 512] as one [128, 4, 512] tile (partition p
    holds rows 4p..4p+3 -> contiguous 8KB descriptors, the per-engine
    throughput sweet spot: ~26.5 GB/s x 16 engines ~ 424 GB/s).
  - Threshold: ACT computes h = Relu(x*2^25 - (2^24-1)) in bf16, which
    is exactly 0 iff x < 0.5 and >= 1 otherwise (exact for every f32).
  - Column extents: one-hot [128, 16] lhsT matmuls accumulate per-image
    column hit-mass into PSUM [16, 512] (partition = image); gpsimd
    pre-adds block pairs to halve the matmul count; then compare/mul/
    reduce on DVE.
  - Row extents stay in [128, *] space: rowmax over W of h (bf16)
    -> [128, 64] (col = i*4 + b, image row r = 4p + b), compare, mul
    by index consts, reduce over b -> [128, 16] (col = image), one PE
    transpose -> [16, 128] PSUM, full-partition reduce. This chain
    runs on gpsimd at the tail, in parallel with the X chain on DVE.
  - Last image: two [128, 2, 512] half loads (4KB descriptors) so the
    final arrival -> answer chain is short.
"""

import numpy as np
import ml_dtypes
from contextlib import ExitStack

import concourse.bass as bass
import concourse.bacc as bacc
import concourse.tile as tile
import concourse.mybir as mybir
from concourse.bass_utils import run_bass_kernel_spmd

N_CORES = 8
N, H, W = 128, 512, 512
NPC = N // N_CORES          # images per core = 16
P = 128                     # SBUF partitions
NBLK = H // P               # 4 row blocks per image
F32 = mybir.dt.float32
BF16 = mybir.dt.bfloat16
I32 = mybir.dt.int32

# Relu(x * 2^25 - (2^24 - 1)) == 0 iff x < 0.5, >= 1 iff x >= 0.5, exact
# for EVERY f32 x: x*2^25 is exact (power-of-2 scale); for x < 0.5,
# x*2^25 <= 2^24 - 1 so the true sum is <= 0 (rounding is monotone, 0 is
# representable); for x >= 0.5 the true sum is >= 1 and rounds to >= 1.
ACT_SCALE = float(2**25)
ACT_BIAS = float(1 - 2**24)

TRACE = False               # test.py sets True to capture a HW profile
LAST_RESULTS = None         # BassKernelResults of the last run

_compiled = None


def _build_nc():
    nc = bacc.Bacc(
        "TRN2", target_bir_lowering=False, debug=False, num_devices=N_CORES
    )
    mask_d = nc.dram_tensor("mask", [NPC * H, W], F32, kind="ExternalInput").ap()
    oneh_d = nc.dram_tensor("onehot", [P, NPC * NPC], BF16, kind="ExternalInput").ap()
    # packed f32 consts: ident [0:128] | ylo [128:192] | yhi [192:256]
    pack_d = nc.dram_tensor("cpack", [P, 2 * P], F32, kind="ExternalInput").ap()
    xlo_d = nc.dram_tensor("xlo_const", [NPC, W], F32, kind="ExternalInput").ap()
    xhi_d = nc.dram_tensor("xhi_const", [NPC, W], F32, kind="ExternalInput").ap()
    bbox_d = nc.dram_tensor("bbox", [NPC, 4], I32, kind="ExternalOutput").ap()

    with tile.TileContext(nc) as tc, ExitStack() as ctx:
        consts = ctx.enter_context(tc.tile_pool(name="consts", bufs=1))
        xpool = ctx.enter_context(tc.tile_pool(name="x", bufs=4))
        hpool = ctx.enter_context(tc.tile_pool(name="h", bufs=6))
        hspool = ctx.enter_context(tc.tile_pool(name="hs", bufs=4))
        lastpool = ctx.enter_context(tc.tile_pool(name="last", bufs=2))
        small = ctx.enter_context(tc.tile_pool(name="small", bufs=1))
        scratch = ctx.enter_context(tc.tile_pool(name="scratch", bufs=2))
        psum = ctx.enter_context(tc.tile_pool(name="psum", bufs=1, space="PSUM"))

        # pin const loads to the start of the schedule, all on the SYNC
        # queue: the scalar queue stays completely idle, so the DMA
        # queue-manager engine (which also carries 1/16 of the mask and
        # is the stream straggler) never juggles two descriptor streams.
        # Packed shapes keep the const descriptor count tiny. The tile
        # scheduler otherwise sinks tail-only consts next to their
        # consumers, adding their DMA latency to the tail.
        with tc.high_priority():
            oneh = consts.tile([P, NPC * NPC], BF16)
            nc.scalar.dma_start(out=oneh[:], in_=oneh_d)
            cpack = consts.tile([P, 2 * P], F32)
            nc.scalar.dma_start(out=cpack[:], in_=pack_d)
            xlo_c = consts.tile([NPC, W], F32)
            nc.scalar.dma_start(out=xlo_c[:], in_=xlo_d)
            xhi_c = consts.tile([NPC, W], F32)
            nc.scalar.dma_start(out=xhi_c[:], in_=xhi_d)
            act_bias = consts.tile([P, 1], F32)
            nc.vector.memset(act_bias[:], ACT_BIAS)
        ident = cpack[:, 0:P]
        ylo_c = cpack[:, P:P + NPC * NBLK]
        yhi_c = cpack[:, P + NPC * NBLK:2 * P]

        # rowmax[p, i*4 + b]: any-hit indicator per image row r = 4p + b.
        # Images 0-13: DVE max over w of h. Images 14/15: ACT accum_out
        # row SUMS (sum of relu-mass: 0 iff no hit) — frees DVE's tail.
        # Both are 0 iff no hit and >= 1 otherwise, so one is_ge works.
        rowmax = small.tile([P, NPC * NBLK], F32)
        rowmax_v = rowmax.rearrange("p (i b) -> p i b", i=NPC)
        cnt_ps = psum.tile([NPC, W], F32)    # per-image column hit-mass
        tpsL = psum.tile([NPC, P], F32)      # transposed y-lo stage
        tpsH = psum.tile([NPC, P], F32)      # transposed y-hi stage

        # images 0-13 as 7 DUAL-image DMA instructions (256 x 8KB
        # descriptors each): halves the mask instruction count — the
        # queue-manager DMA engine (the stream straggler) pays a
        # per-instruction completion overhead, so fewer instructions
        # pull in the stream tail
        for t in range(7):
            x = xpool.tile([P, 2, NBLK, W], F32, tag="x")
            nc.sync.dma_start(
                out=x[:],
                in_=mask_d[2 * t * H:(2 * t + 2) * H, :]
                .rearrange("(i p b) w -> p i b w", i=2, p=P),
            )
            h = hpool.tile([P, 2, NBLK, W], BF16, tag="h")
            for j in range(2):
                nc.scalar.activation(
                    h[:, j], x[:, j], mybir.ActivationFunctionType.Relu,
                    bias=act_bias[:], scale=ACT_SCALE,
                )
                nc.vector.tensor_reduce(
                    out=rowmax_v[:, 2 * t + j, :], in_=h[:, j],
                    axis=mybir.AxisListType.X, op=mybir.AluOpType.max,
                )
            if t < 6:
                # pre-sum block pairs on gpsimd, both images in one op:
                # halves the PE matmul count (hit-mass stays 0 iff no hit)
                h_v = h.rearrange("p i (m q) w -> p i m q w", q=2)
                hs = hspool.tile([P, 2, 2, W], BF16)
                nc.gpsimd.tensor_add(hs[:], h_v[:, :, :, 0, :], h_v[:, :, :, 1, :])
                for j in range(2):
                    lhsT = oneh[:, (2 * t + j) * NPC:(2 * t + j + 1) * NPC]
                    for m in range(2):
                        nc.tensor.matmul(
                            cnt_ps[:, :], lhsT, hs[:, j, m, :],
                            start=(t == 0 and j == 0 and m == 0), stop=False,
                        )
            else:
                # last dual (images 12,13): direct matmuls — the 4us
                # gpsimd hop sits on the tail critical path (PE program
                # order makes images 14/15's matmuls queue behind it)
                for j in range(2):
                    lhsT = oneh[:, (2 * t + j) * NPC:(2 * t + j + 1) * NPC]
                    for b in range(NBLK):
                        nc.tensor.matmul(
                            cnt_ps[:, :], lhsT, h[:, j, b, :],
                            start=False, stop=False,
                        )

        # image 14: single-image DMA into half a dual tile (stays on the
        # xpool ring so its trigger is paced like the duals), per-block
        # acts with accum_out row sums (no DVE rowmax), direct matmuls
        i = NPC - 2
        x = xpool.tile([P, 2, NBLK, W], F32, tag="x")
        nc.sync.dma_start(
            out=x[:, 0],
            in_=mask_d[i * H:(i + 1) * H, :].rearrange("(p b) w -> p b w", p=P),
        )
        h = hpool.tile([P, 2, NBLK, W], BF16, tag="h")
        # threshold image 14 on gpsimd (idle after the last pre-add) so
        # the Scalar engine can start image 15's blocks immediately; a
        # direct is_ge compare is exact, and the 0/1 hit counts keep the
        # cnt >= 0.5 semantics
        nc.gpsimd.tensor_scalar(
            h[:, 0], x[:, 0], 0.5, None, mybir.AluOpType.is_ge
        )
        # rowmax(14) on DVE, which is idle in this window
        nc.vector.tensor_reduce(
            out=rowmax_v[:, i, :], in_=h[:, 0],
            axis=mybir.AxisListType.X, op=mybir.AluOpType.max,
        )
        lhsT = oneh[:, i * NPC:(i + 1) * NPC]
        for b in range(NBLK):
            nc.tensor.matmul(
                cnt_ps[:, :], lhsT, h[:, 0, b, :],
                start=False, stop=False,
            )

        # image 15: two half loads so its compute chain starts while
        # the second half is still in flight; per-block acts + accum
        i = NPC - 1
        lhsT = oneh[:, i * NPC:(i + 1) * NPC]
        for u in range(2):
            x = lastpool.tile([P, 2, W], F32, tag="xh")
            nc.sync.dma_start(
                out=x[:],
                in_=mask_d[i * H:(i + 1) * H, :]
                .rearrange("(p b) w -> p b w", p=P)[:, 2 * u:2 * u + 2, :],
            )
            h = lastpool.tile([P, 2, W], BF16, tag="hh")
            for b in range(2):
                nc.scalar.activation(
                    h[:, b:b + 1, :], x[:, b:b + 1, :],
                    mybir.ActivationFunctionType.Relu,
                    bias=act_bias[:], scale=ACT_SCALE,
                    accum_out=rowmax_v[:, i, 2 * u + b:2 * u + b + 1],
                )
                nc.tensor.matmul(
                    cnt_ps[:, :], lhsT, h[:, b, :],
                    start=False, stop=(u == 1 and b == 1),
                )

        # raw extents tile: col 0 = ylo, 1 = xlo, 2 = yhi, 3 = xhi
        # (lo values are lo-512 for hit, 0 for none; hi are hi+1 or 0)
        raw = small.tile([NPC, 4], F32)

        # ---- X extents from cnt_ps [16, 512] on DVE (issued first so
        # the DVE starts X as soon as the stop-matmul lands) ----
        # NOTE: tensor_tensor_reduce and scalar_tensor_tensor (fused DVE
        # ISA ops) both crash the exec unit on this runtime path; use
        # plain compare/mul + reduce.
        colhit = small.tile([NPC, W], F32)
        nc.vector.tensor_scalar(
            colhit[:], cnt_ps[:], 0.5, None, mybir.AluOpType.is_ge
        )
        xprod = scratch.tile([NPC, W], F32, tag="xprod")
        nc.vector.tensor_mul(xprod[:], colhit[:], xlo_c[:])
        nc.vector.tensor_reduce(
            out=raw[:, 1:2], in_=xprod[:],
            axis=mybir.AxisListType.X, op=mybir.AluOpType.min,
        )
        xprod2 = scratch.tile([NPC, W], F32, tag="xprod")
        nc.vector.tensor_mul(xprod2[:], colhit[:], xhi_c[:])
        nc.vector.tensor_reduce(
            out=raw[:, 3:4], in_=xprod2[:],
            axis=mybir.AxisListType.X, op=mybir.AluOpType.max,
        )

        # ---- Y extents in [128, *] space; compare+mul on gpsimd run
        # concurrently with the DVE X chain ----
        rowhit = small.tile([P, NPC * NBLK], F32)
        nc.gpsimd.tensor_scalar(
            rowhit[:], rowmax[:], 0.5, None, mybir.AluOpType.is_ge
        )
        # loI[:, i] = min over b of rowhit*(r-512); 0 if no hit (and 0
        # is neutral for the final min since hits give negatives)
        loI = small.tile([P, NPC], F32)
        hiI = small.tile([P, NPC], F32)
        prod = scratch.tile([P, NPC * NBLK], F32, tag="yprod")
        nc.gpsimd.tensor_mul(prod[:], rowhit[:], ylo_c)
        prod_v = prod.rearrange("p (i b) -> p i b", i=NPC)
        nc.vector.tensor_reduce(
            out=loI[:], in_=prod_v[:],
            axis=mybir.AxisListType.X, op=mybir.AluOpType.min,
        )
        prod2 = scratch.tile([P, NPC * NBLK], F32, tag="yprod")
        nc.gpsimd.tensor_mul(prod2[:], rowhit[:], yhi_c)
        prod2_v = prod2.rearrange("p (i b) -> p i b", i=NPC)
        nc.vector.tensor_reduce(
            out=hiI[:], in_=prod2_v[:],
            axis=mybir.AxisListType.X, op=mybir.AluOpType.max,
        )
        # transpose [128, 16] -> [16, 128], partition = image
        nc.tensor.matmul(
            tpsL[:, :], loI[:], ident,
            is_transpose=True, start=True, stop=True,
        )
        nc.tensor.matmul(
            tpsH[:, :], hiI[:], ident,
            is_transpose=True, start=True, stop=True,
        )
        nc.vector.tensor_reduce(
            out=raw[:, 0:1], in_=tpsL[:, :],
            axis=mybir.AxisListType.X, op=mybir.AluOpType.min,
        )
        nc.vector.tensor_reduce(
            out=raw[:, 2:3], in_=tpsH[:, :],
            axis=mybir.AxisListType.X, op=mybir.AluOpType.max,
        )

        # fixup: gm = (hi_raw > 0) * 512 (hit indicator scaled);
        # lo_final = lo_raw + gm   (hit: ymin-512+512 = ymin; none: 0)
        # hi_final = hi_raw + 512 - gm  (hit: hi_raw; none: 512)
        bbox_f = small.tile([NPC, 4], F32)
        gm = small.tile([NPC, 2], F32)
        nc.vector.tensor_scalar(
            gm[:], raw[:, 2:4], 0.0, float(H),
            mybir.AluOpType.is_gt, mybir.AluOpType.mult,
        )
        nc.vector.tensor_add(bbox_f[:, 0:2], raw[:, 0:2], gm[:])
        t5 = small.tile([NPC, 2], F32)
        nc.vector.tensor_scalar_add(t5[:], raw[:, 2:4], float(H))
        nc.vector.tensor_sub(bbox_f[:, 2:4], t5[:], gm[:])

        bbox_i = small.tile([NPC, 4], I32)
        nc.vector.tensor_copy(bbox_i[:], bbox_f[:])
        nc.sync.dma_start(out=bbox_d, in_=bbox_i[:])

    nc.compile()
    return nc


def _consts():
    oneh = np.zeros((P, NPC * NPC), dtype=ml_dtypes.bfloat16)
    for i in range(NPC):
        oneh[:, i * NPC + i] = 1.0
    ident = np.eye(P, dtype=np.float32)
    f = np.arange(W, dtype=np.float32)
    xlo = np.broadcast_to(f - W, (NPC, W)).copy()
    xhi = np.broadcast_to(f + 1, (NPC, W)).copy()
    # block b on partition p is image row r = 4p + b, layout (i b)
    p = np.arange(P)
    b = np.arange(NBLK)
    r = (NBLK * p[:, None] + b[None, :]).astype(np.float32)  # [128, 4]
    ylo = np.tile(r - H, (1, NPC)).astype(np.float32)
    yhi = np.tile(r + 1, (1, NPC)).astype(np.float32)
    pack = np.concatenate([ident, ylo, yhi], axis=1).astype(np.float32)
    return oneh, pack, xlo, xhi


def kernel(mask):
    global _compiled, LAST_RESULTS
    mask = np.ascontiguousarray(np.asarray(mask), dtype=np.float32)
    assert mask.shape == (N, 1, H, W), mask.shape
    if _compiled is None:
        _compiled = _build_nc()
    nc = _compiled
    oneh, pack, xlo, xhi = _consts()
    m = mask.reshape(N, H, W)
    in_maps = []
    for c in range(N_CORES):
        in_maps.append({
            "mask": np.ascontiguousarray(
                m[c * NPC:(c + 1) * NPC].reshape(NPC * H, W)
            ),
            "onehot": oneh,
            "cpack": pack,
            "xlo_const": xlo,
            "xhi_const": xhi,
        })
    res = run_bass_kernel_spmd(nc, in_maps, list(range(N_CORES)), trace=TRACE)
    LAST_RESULTS = res
    out = np.concatenate([res.results[c]["bbox"] for c in range(N_CORES)], axis=0)
    return out.astype(np.int32, copy=False)


# revision 42
# speedup vs baseline: 1.4751x; 1.4751x over previous
"""Bounding-box kernel for Trainium2 (Bass/Tile), 8-core SPMD.

Problem: mask [128, 1, 512, 512] f32 -> bbox [128, 4] int32
  (y_min, x_min, y_max, x_max) of the region where mask >= 0.5,
  with (0, 0, H, W) when a row/col has no hit.

Strategy (per core, 16 images, single qSync HWDGE queue):
  - DMA each image [512, 512] as one [128, 4, 512] tile (partition p
    holds rows 4p..4p+3 -> contiguous 8KB descriptors, the per-engine
    throughput sweet spot: ~26.5 GB/s x 16 engines ~ 424 GB/s).
  - Threshold: ACT computes h = Relu(x*2^25 - (2^24-1)) in bf16, which
    is exactly 0 iff x < 0.5 and >= 1 otherwise (exact for every f32).
  - Column extents: one-hot [128, 16] lhsT matmuls accumulate per-image
    column hit-mass into PSUM [16, 512] (partition = image); gpsimd
    pre-adds block pairs to halve the matmul count; then compare/mul/
    reduce on DVE.
  - Row extents stay in [128, *] space: rowmax over W of h (bf16)
    -> [128, 64] (col = i*4 + b, image row r = 4p + b), compare, mul
    by index consts, reduce over b -> [128, 16] (col = image), one PE
    transpose -> [16, 128] PSUM, full-partition reduce. This chain
    runs on gpsimd at the tail, in parallel with the X chain on DVE.
  - Last image: two [128, 2, 512] half loads (4KB descriptors) so the
    final arrival -> answer chain is short.
"""

import numpy as np
import ml_dtypes
from contextlib import ExitStack

import concourse.bass as bass
import concourse.bacc as bacc
import concourse.tile as tile
import concourse.mybir as mybir
from concourse.bass_utils import run_bass_kernel_spmd

N_CORES = 8
N, H, W = 128, 512, 512
NPC = N // N_CORES          # images per core = 16
P = 128                     # SBUF partitions
NBLK = H // P               # 4 row blocks per image
F32 = mybir.dt.float32
BF16 = mybir.dt.bfloat16
I32 = mybir.dt.int32

# Relu(x * 2^25 - (2^24 - 1)) == 0 iff x < 0.5, >= 1 iff x >= 0.5, exact
# for EVERY f32 x: x*2^25 is exact (power-of-2 scale); for x < 0.5,
# x*2^25 <= 2^24 - 1 so the true sum is <= 0 (rounding is monotone, 0 is
# representable); for x >= 0.5 the true sum is >= 1 and rounds to >= 1.
ACT_SCALE = float(2**25)
ACT_BIAS = float(1 - 2**24)

TRACE = False               # test.py sets True to capture a HW profile
LAST_RESULTS = None         # BassKernelResults of the last run

_compiled = None


def _build_nc():
    nc = bacc.Bacc(
        "TRN2", target_bir_lowering=False, debug=False, num_devices=N_CORES
    )
    mask_d = nc.dram_tensor("mask", [NPC * H, W], F32, kind="ExternalInput").ap()
    oneh_d = nc.dram_tensor("onehot", [P, NPC * NPC], BF16, kind="ExternalInput").ap()
    # packed f32 consts: ident [0:128] | ylo [128:192] | yhi [192:256]
    pack_d = nc.dram_tensor("cpack", [P, 2 * P], F32, kind="ExternalInput").ap()
    xlo_d = nc.dram_tensor("xlo_const", [NPC, W], F32, kind="ExternalInput").ap()
    xhi_d = nc.dram_tensor("xhi_const", [NPC, W], F32, kind="ExternalInput").ap()
    bbox_d = nc.dram_tensor("bbox", [NPC, 4], I32, kind="ExternalOutput").ap()

    with tile.TileContext(nc) as tc, ExitStack() as ctx:
        consts = ctx.enter_context(tc.tile_pool(name="consts", bufs=1))
        xpool = ctx.enter_context(tc.tile_pool(name="x", bufs=4))
        hpool = ctx.enter_context(tc.tile_pool(name="h", bufs=6))
        hspool = ctx.enter_context(tc.tile_pool(name="hs", bufs=4))
        lastpool = ctx.enter_context(tc.tile_pool(name="last", bufs=2))
        small = ctx.enter_context(tc.tile_pool(name="small", bufs=1))
        scratch = ctx.enter_context(tc.tile_pool(name="scratch", bufs=2))
        psum = ctx.enter_context(tc.tile_pool(name="psum", bufs=1, space="PSUM"))

        # pin const loads to the start of the schedule, all on the SYNC
        # queue: the scalar queue stays completely idle, so the DMA
        # queue-manager engine (which also carries 1/16 of the mask and
        # is the stream straggler) never juggles two descriptor streams.
        # Packed shapes keep the const descriptor count tiny. The tile
        # scheduler otherwise sinks tail-only consts next to their
        # consumers, adding their DMA latency to the tail.
        with tc.high_priority():
            oneh = consts.tile([P, NPC * NPC], BF16)
            nc.scalar.dma_start(out=oneh[:], in_=oneh_d)
            cpack = consts.tile([P, 2 * P], F32)
            nc.scalar.dma_start(out=cpack[:], in_=pack_d)
            xlo_c = consts.tile([NPC, W], F32)
            nc.scalar.dma_start(out=xlo_c[:], in_=xlo_d)
            xhi_c = consts.tile([NPC, W], F32)
            nc.scalar.dma_start(out=xhi_c[:], in_=xhi_d)
            act_bias = consts.tile([P, 1], F32)
            nc.vector.memset(act_bias[:], ACT_BIAS)
        ident = cpack[:, 0:P]
        ylo_c = cpack[:, P:P + NPC * NBLK]
        yhi_c = cpack[:, P + NPC * NBLK:2 * P]

        # rowmax[p, i*4 + b]: any-hit indicator per image row r = 4p + b.
        # Images 0-13: DVE max over w of h. Images 14/15: ACT accum_out
        # row SUMS (sum of relu-mass: 0 iff no hit) — frees DVE's tail.
        # Both are 0 iff no hit and >= 1 otherwise, so one is_ge works.
        rowmax = small.tile([P, NPC * NBLK], F32)
        rowmax_v = rowmax.rearrange("p (i b) -> p i b", i=NPC)
        cnt_ps = psum.tile([NPC, W], F32)    # per-image column hit-mass
        tpsL = psum.tile([NPC, P], F32)      # transposed y-lo stage
        tpsH = psum.tile([NPC, P], F32)      # transposed y-hi stage

        # images 0-13 as 7 DUAL-image DMA instructions (256 x 8KB
        # descriptors each): halves the mask instruction count — the
        # queue-manager DMA engine (the stream straggler) pays a
        # per-instruction completion overhead, so fewer instructions
        # pull in the stream tail
        for t in range(7):
            x = xpool.tile([P, 2, NBLK, W], F32, tag="x")
            nc.sync.dma_start(
                out=x[:],
                in_=mask_d[2 * t * H:(2 * t + 2) * H, :]
                .rearrange("(i p b) w -> p i b w", i=2, p=P),
            )
            h = hpool.tile([P, 2, NBLK, W], BF16, tag="h")
            for j in range(2):
                nc.scalar.activation(
                    h[:, j], x[:, j], mybir.ActivationFunctionType.Relu,
                    bias=act_bias[:], scale=ACT_SCALE,
                )
                nc.vector.tensor_reduce(
                    out=rowmax_v[:, 2 * t + j, :], in_=h[:, j],
                    axis=mybir.AxisListType.X, op=mybir.AluOpType.max,
                )
            if t < 6:
                # pre-sum block pairs on gpsimd, both images in one op:
                # halves the PE matmul count (hit-mass stays 0 iff no hit)
                h_v = h.rearrange("p i (m q) w -> p i m q w", q=2)
                hs = hspool.tile([P, 2, 2, W], BF16)
                nc.gpsimd.tensor_add(hs[:], h_v[:, :, :, 0, :], h_v[:, :, :, 1, :])
                for j in range(2):
                    lhsT = oneh[:, (2 * t + j) * NPC:(2 * t + j + 1) * NPC]
                    for m in range(2):
                        nc.tensor.matmul(
                            cnt_ps[:, :], lhsT, hs[:, j, m, :],
                            start=(t == 0 and j == 0 and m == 0), stop=False,
                        )
            else:
                # last dual (images 12,13): direct matmuls — the 4us
                # gpsimd hop sits on the tail critical path (PE program
                # order makes images 14/15's matmuls queue behind it)
                for j in range(2):
                    lhsT = oneh[:, (2 * t + j) * NPC:(2 * t + j + 1) * NPC]
                    for b in range(NBLK):
                        nc.tensor.matmul(
                            cnt_ps[:, :], lhsT, h[:, j, b, :],
                            start=False, stop=False,
                        )

        # image 14: single-image DMA into half a dual tile (stays on the
        # xpool ring so its trigger is paced like the duals), per-block
        # acts with accum_out row sums (no DVE rowmax), direct matmuls
        i = NPC - 2
        x = xpool.tile([P, 2, NBLK, W], F32, tag="x")
        nc.sync.dma_start(
            out=x[:, 0],
            in_=mask_d[i * H:(i + 1) * H, :].rearrange("(p b) w -> p b w", p=P),
        )
        h = hpool.tile([P, 2, NBLK, W], BF16, tag="h")
        lhsT = oneh[:, i * NPC:(i + 1) * NPC]
        for b in range(NBLK):
            nc.scalar.activation(
                h[:, 0, b:b + 1, :], x[:, 0, b:b + 1, :],
                mybir.ActivationFunctionType.Relu,
                bias=act_bias[:], scale=ACT_SCALE,
                accum_out=rowmax_v[:, i, b:b + 1],
            )
            nc.tensor.matmul(
                cnt_ps[:, :], lhsT, h[:, 0, b, :],
                start=False, stop=False,
            )

        # image 15: two half loads so its compute chain starts while
        # the second half is still in flight; per-block acts + accum
        i = NPC - 1
        lhsT = oneh[:, i * NPC:(i + 1) * NPC]
        for u in range(2):
            x = lastpool.tile([P, 2, W], F32, tag="xh")
            nc.sync.dma_start(
                out=x[:],
                in_=mask_d[i * H:(i + 1) * H, :]
                .rearrange("(p b) w -> p b w", p=P)[:, 2 * u:2 * u + 2, :],
            )
            h = lastpool.tile([P, 2, W], BF16, tag="hh")
            for b in range(2):
                nc.scalar.activation(
                    h[:, b:b + 1, :], x[:, b:b + 1, :],
                    mybir.ActivationFunctionType.Relu,
                    bias=act_bias[:], scale=ACT_SCALE,
                    accum_out=rowmax_v[:, i, 2 * u + b:2 * u + b + 1],
                )
                nc.tensor.matmul(
                    cnt_ps[:, :], lhsT, h[:, b, :],
                    start=False, stop=(u == 1 and b == 1),
                )

        # raw extents tile: col 0 = ylo, 1 = xlo, 2 = yhi, 3 = xhi
        # (lo values are lo-512 for hit, 0 for none; hi are hi+1 or 0)
        raw = small.tile([NPC, 4], F32)

        # ---- X extents from cnt_ps [16, 512] on DVE (issued first so
        # the DVE starts X as soon as the stop-matmul lands) ----
        # NOTE: tensor_tensor_reduce and scalar_tensor_tensor (fused DVE
        # ISA ops) both crash the exec unit on this runtime path; use
        # plain compare/mul + reduce.
        colhit = small.tile([NPC, W], F32)
        nc.vector.tensor_scalar(
            colhit[:], cnt_ps[:], 0.5, None, mybir.AluOpType.is_ge
        )
        xprod = scratch.tile([NPC, W], F32, tag="xprod")
        nc.vector.tensor_mul(xprod[:], colhit[:], xlo_c[:])
        nc.vector.tensor_reduce(
            out=raw[:, 1:2], in_=xprod[:],
            axis=mybir.AxisListType.X, op=mybir.AluOpType.min,
        )
        xprod2 = scratch.tile([NPC, W], F32, tag="xprod")
        nc.vector.tensor_mul(xprod2[:], colhit[:], xhi_c[:])
        nc.vector.tensor_reduce(
            out=raw[:, 3:4], in_=xprod2[:],
            axis=mybir.AxisListType.X, op=mybir.AluOpType.max,
        )

        # ---- Y extents in [128, *] space; compare+mul on gpsimd run
        # concurrently with the DVE X chain ----
        rowhit = small.tile([P, NPC * NBLK], F32)
        # on DVE: gpsimd compares run ~17.7ns/elem (1.1us here) and
        # delay the Y muls into DVE's X window; DVE does this in ~190ns
        # during its idle slot right after the last rowmax
        nc.vector.tensor_scalar(
            rowhit[:], rowmax[:], 0.5, None, mybir.AluOpType.is_ge
        )
        # loI[:, i] = min over b of rowhit*(r-512); 0 if no hit (and 0
        # is neutral for the final min since hits give negatives)
        loI = small.tile([P, NPC], F32)
        hiI = small.tile([P, NPC], F32)
        prod = scratch.tile([P, NPC * NBLK], F32, tag="yprod")
        nc.gpsimd.tensor_mul(prod[:], rowhit[:], ylo_c)
        prod_v = prod.rearrange("p (i b) -> p i b", i=NPC)
        nc.vector.tensor_reduce(
            out=loI[:], in_=prod_v[:],
            axis=mybir.AxisListType.X, op=mybir.AluOpType.min,
        )
        prod2 = scratch.tile([P, NPC * NBLK], F32, tag="yprod")
        nc.gpsimd.tensor_mul(prod2[:], rowhit[:], yhi_c)
        prod2_v = prod2.rearrange("p (i b) -> p i b", i=NPC)
        nc.vector.tensor_reduce(
            out=hiI[:], in_=prod2_v[:],
            axis=mybir.AxisListType.X, op=mybir.AluOpType.max,
        )
        # transpose [128, 16] -> [16, 128], partition = image
        nc.tensor.matmul(
            tpsL[:, :], loI[:], ident,
            is_transpose=True, start=True, stop=True,
        )
        nc.tensor.matmul(
            tpsH[:, :], hiI[:], ident,
            is_transpose=True, start=True, stop=True,
        )
        nc.vector.tensor_reduce(
            out=raw[:, 0:1], in_=tpsL[:, :],
            axis=mybir.AxisListType.X, op=mybir.AluOpType.min,
        )
        nc.vector.tensor_reduce(
            out=raw[:, 2:3], in_=tpsH[:, :],
            axis=mybir.AxisListType.X, op=mybir.AluOpType.max,
        )

        # fixup: gm = (hi_raw > 0) * 512 (hit indicator scaled);
        # lo_final = lo_raw + gm   (hit: ymin-512+512 = ymin; none: 0)
        # hi_final = hi_raw + 512 - gm  (hit: hi_raw; none: 512)
        bbox_f = small.tile([NPC, 4], F32)
        gm = small.tile([NPC, 2], F32)
        nc.vector.tensor_scalar(
            gm[:], raw[:, 2:4], 0.0, float(H),
            mybir.AluOpType.is_gt, mybir.AluOpType.mult,
        )
        nc.vector.tensor_add(bbox_f[:, 0:2], raw[:, 0:2], gm[:])
        t5 = small.tile([NPC, 2], F32)
        nc.vector.tensor_scalar_add(t5[:], raw[:, 2:4], float(H))
        nc.vector.tensor_sub(bbox_f[:, 2:4], t5[:], gm[:])

        bbox_i = small.tile([NPC, 4], I32)
        nc.vector.tensor_copy(bbox_i[:], bbox_f[:])
        nc.sync.dma_start(out=bbox_d, in_=bbox_i[:])

    nc.compile()
    return nc


def _consts():
    oneh = np.zeros((P, NPC * NPC), dtype=ml_dtypes.bfloat16)
    for i in range(NPC):
        oneh[:, i * NPC + i] = 1.0
    ident = np.eye(P, dtype=np.float32)
    f = np.arange(W, dtype=np.float32)
    xlo = np.broadcast_to(f - W, (NPC, W)).copy()
    xhi = np.broadcast_to(f + 1, (NPC, W)).copy()
    # block b on partition p is image row r = 4p + b, layout (i b)
    p = np.arange(P)
    b = np.arange(NBLK)
    r = (NBLK * p[:, None] + b[None, :]).astype(np.float32)  # [128, 4]
    ylo = np.tile(r - H, (1, NPC)).astype(np.float32)
    yhi = np.tile(r + 1, (1, NPC)).astype(np.float32)
    pack = np.concatenate([ident, ylo, yhi], axis=1).astype(np.float32)
    return oneh, pack, xlo, xhi


def kernel(mask):
    global _compiled, LAST_RESULTS
    mask = np.ascontiguousarray(np.asarray(mask), dtype=np.float32)
    assert mask.shape == (N, 1, H, W), mask.shape
    if _compiled is None:
        _compiled = _build_nc()
    nc = _compiled
    oneh, pack, xlo, xhi = _consts()
    m = mask.reshape(N, H, W)
    in_maps = []
    for c in range(N_CORES):
        in_maps.append({
            "mask": np.ascontiguousarray(
                m[c * NPC:(c + 1) * NPC].reshape(NPC * H, W)
            ),
            "onehot": oneh,
            "cpack": pack,
            "xlo_const": xlo,
            "xhi_const": xhi,
        })
    res = run_bass_kernel_spmd(nc, in_maps, list(range(N_CORES)), trace=TRACE)
    LAST_RESULTS = res
    out = np.concatenate([res.results[c]["bbox"] for c in range(N_CORES)], axis=0)
    return out.astype(np.int32, copy=False)
